# revision 3
# baseline (speedup 1.0000x reference)
"""Cross-image contrastive loss on 8 TRN2 NeuronCores.

Strategy (row-parallel over the N=4096 pixel dim, 512 rows per core):
  - The label mask for diff_sum is folded into the matmul contraction:
    augmented K = d + L + 1 = 84 with [Fi; onehot_lab; 1]^T [Fjj; C*onehot_jj; -C],
    so masked logits come out of a single matmul and both row reductions
    (sum_s1 and diff_sum) are fused exp+row-sum on the Scalar engine
    (activation accum_out).
  - bf16 matmul inputs (PE 1 cyc/row vs 4 for f32); f32 PSUM accumulation.
  - diag terms, label histogram, per-label weights all computed on device.
  - Each core emits its partial loss; host sums the 8 partials.
"""

import sys

import numpy as np

sys.path.insert(0, "/opt/trn_rl_repo")

import ml_dtypes

TAU = 0.07
EPS = 1e-4
L = 19
D = 64
N = 4096
NCORES = 8
P = N // NCORES  # 512 rows per core
KA = D + L + 1  # 84 augmented contraction
CMASK = 4.25  # bf16-exact mask magnitude; CMASK/TAU ~ 60.7 in the exponent
PB = P // 128  # 4 partition blocks per core

_compiled = None


def _build():
    from concourse import bacc, mybir, tile

    f32 = mybir.dt.float32
    bf16 = mybir.dt.bfloat16
    Exp = mybir.ActivationFunctionType.Exp
    Ln = mybir.ActivationFunctionType.Ln
    X = mybir.AxisListType.X
    add = mybir.AluOpType.add

    nc = bacc.Bacc("TRN2", target_bir_lowering=False, debug=False)

    lhs_d = nc.dram_tensor("lhs", (KA, P), bf16, kind="ExternalInput")
    rhs_d = nc.dram_tensor("rhs", (KA, 2 * N), bf16, kind="ExternalInput")
    fiT_d = nc.dram_tensor("fiT", (128, PB * D), f32, kind="ExternalInput")
    fsT_d = nc.dram_tensor("fsT", (128, PB * D), f32, kind="ExternalInput")
    ohcnt_d = nc.dram_tensor("ohcnt", (L, 2 * N), f32, kind="ExternalInput")
    ohlab_d = nc.dram_tensor("ohlab", (L, P), f32, kind="ExternalInput")
    ones_d = nc.dram_tensor("ones", (128, 1), f32, kind="ExternalInput")
    out_d = nc.dram_tensor("out", (1, 1), f32, kind="ExternalOutput")

    NCHUNK = 16  # 8 S1 + 8 S2 chunks of 512 columns
    NG = 4  # psum groups per p-block (4 chunks each)

    with tile.TileContext(nc) as tc:
        with (
            tc.tile_pool(name="res", bufs=1) as res,
            tc.tile_pool(name="scr", bufs=2) as scr,
            tc.tile_pool(name="ps", bufs=2, space="PSUM") as psp,
        ):
            # ---- resident SBUF tensors ----
            lhs_sb = res.tile([KA, P], bf16, tag="lhs")
            rhs_sb = res.tile([KA, 2 * N], bf16, tag="rhs")
            fiT_sb = res.tile([128, PB * D], f32, tag="fiT")
            fsT_sb = res.tile([128, PB * D], f32, tag="fsT")
            ohcnt_sb = res.tile([L, 2 * N], f32, tag="ohcnt")
            ohlab_sb = res.tile([L, P], f32, tag="ohlab")
            ones_sb = res.tile([128, 1], f32, tag="ones")
            acc = res.tile([128, 16], f32, tag="acc")  # col = g*4 + b

            nc.sync.dma_start(lhs_sb[:], lhs_d[:])
            for ch in range(NCHUNK):
                nc.sync.dma_start(
                    rhs_sb[:, ch * 512 : (ch + 1) * 512],
                    rhs_d[:, ch * 512 : (ch + 1) * 512],
                )
            nc.sync.dma_start(fiT_sb[:], fiT_d[:])
            nc.sync.dma_start(fsT_sb[:], fsT_d[:])
            nc.sync.dma_start(ohcnt_sb[:], ohcnt_d[:])
            nc.sync.dma_start(ohlab_sb[:], ohlab_d[:])
            nc.sync.dma_start(ones_sb[:], ones_d[:])

            # ---- main S1/S2 pass: matmul -> exp + row-sum ----
            for b in range(PB):
                for g in range(NG):
                    ps = psp.tile([128, 2048], f32, tag="mm")
                    for c in range(4):
                        ch = g * 4 + c
                        nc.tensor.matmul(
                            ps[:, c * 512 : (c + 1) * 512],
                            lhs_sb[:, b * 128 : (b + 1) * 128],
                            rhs_sb[:, ch * 512 : (ch + 1) * 512],
                            start=True,
                            stop=True,
                        )
                    dump = scr.tile([128, 2048], bf16, tag="dump")
                    nc.scalar.activation(
                        dump[:],
                        ps[:],
                        Exp,
                        scale=1.0 / TAU,
                        accum_out=acc[:, g * 4 + b : g * 4 + b + 1],
                    )

            # ---- label histogram -> per-label weight column ----
            cnt = res.tile([L, 2], f32, tag="cnt")
            nc.vector.tensor_reduce(
                cnt[:], ohcnt_sb[:].rearrange("p (t q) -> p t q", q=N), axis=X, op=add
            )
            dn = res.tile([L, 1], f32, tag="dn")
            nc.vector.tensor_add(dn[:], cnt[:, 0:1], cnt[:, 1:2])
            nc.vector.tensor_scalar_add(dn[:], dn[:], EPS)
            rec = res.tile([L, 1], f32, tag="rec")
            nc.vector.reciprocal(rec[:], dn[:])
            wl = res.tile([L, 1], f32, tag="wl")
            nc.vector.tensor_mul(wl[:], cnt[:, 0:1], rec[:])
            # fold -1/N into the weight so the final reduction is the loss
            nc.vector.tensor_scalar_mul(wl[:], wl[:], -1.0 / N)

            # ---- gather weights to partition-major [128, PB] ----
            wps = psp.tile([128, 2048], f32, tag="mm")
            for b in range(PB):
                nc.tensor.matmul(
                    wps[:, b : b + 1],
                    ohlab_sb[:, b * 128 : (b + 1) * 128],
                    wl[:],
                    start=True,
                    stop=True,
                )
            w_pm = res.tile([128, PB], f32, tag="wpm")
            nc.vector.tensor_copy(w_pm[:], wps[:, 0:PB])

            # ---- A, B, Z, logZ ----
            apm = res.tile([128, PB], f32, tag="apm")
            nc.vector.tensor_add(apm[:], acc[:, 0:4], acc[:, 4:8])
            bpm = res.tile([128, PB], f32, tag="bpm")
            nc.vector.tensor_add(bpm[:], acc[:, 8:12], acc[:, 12:16])
            zpm = res.tile([128, PB], f32, tag="zpm")
            nc.vector.tensor_add(zpm[:], apm[:], bpm[:])
            nc.vector.tensor_scalar_add(zpm[:], zpm[:], EPS)
            logz = res.tile([128, PB], f32, tag="logz")
            nc.scalar.activation(logz[:], zpm[:], Ln)

            # ---- diag = sum_d Fi * (Fii + Fjj), per-64 group sums ----
            prod = res.tile([128, PB * D], f32, tag="prod")
            nc.vector.tensor_mul(prod[:], fiT_sb[:], fsT_sb[:])
            dg = res.tile([128, PB], f32, tag="dg")
            nc.vector.tensor_reduce(
                dg[:],
                prod[:].rearrange("p (b e) -> p b e", e=D),
                axis=X,
                op=add,
            )

            # ---- values = w * (diag/tau - 2*logZ); partial = sum ----
            vals = res.tile([128, PB], f32, tag="vals")
            nc.vector.tensor_scalar_mul(vals[:], dg[:], 1.0 / TAU)
            l2 = res.tile([128, PB], f32, tag="l2")
            nc.vector.tensor_scalar_mul(l2[:], logz[:], -2.0)
            nc.vector.tensor_add(vals[:], vals[:], l2[:])
            nc.vector.tensor_mul(vals[:], vals[:], w_pm[:])
            vred = res.tile([128, 1], f32, tag="vred")
            nc.vector.tensor_reduce(vred[:], vals[:], axis=X, op=add)

            fin = psp.tile([128, 2048], f32, tag="mm")
            nc.tensor.matmul(
                fin[0:1, 0:1], ones_sb[:], vred[:], start=True, stop=True
            )
            res_sb = res.tile([1, 1], f32, tag="res")
            nc.scalar.copy(res_sb[:], fin[0:1, 0:1])
            nc.sync.dma_start(out_d[:], res_sb[:])

    nc.compile()
    return nc


def _make_in_maps(features_i, features_ii, features_jj, i, ii, jj):
    bf16 = ml_dtypes.bfloat16
    Fi = features_i.reshape(D, N).astype(np.float32)
    Fii = features_ii.reshape(D, N).astype(np.float32)
    Fjj = features_jj.reshape(D, N).astype(np.float32)
    lab = i.reshape(-1)
    ii_f = ii.reshape(-1)
    jj_f = jj.reshape(-1)

    lids = np.arange(L, dtype=np.int32)
    oh_jj = (jj_f[None, :] == lids[:, None]).astype(np.float32)  # [L, N]
    oh_ii = (ii_f[None, :] == lids[:, None]).astype(np.float32)

    # rhs (replicated): [KA, 2N] = [S1 | S2]
    rhs = np.zeros((KA, 2 * N), np.float32)
    rhs[0:D, 0:N] = Fii
    rhs[0:D, N:] = Fjj
    rhs[D : D + L, N:] = CMASK * oh_jj
    rhs[D + L, N:] = -CMASK
    rhs = rhs.astype(bf16)

    ohcnt = np.concatenate([oh_ii, oh_jj], axis=1)  # [L, 2N] f32
    ones = np.ones((128, 1), np.float32)

    in_maps = []
    for c in range(NCORES):
        sel = slice(c * P, (c + 1) * P)
        lab_c = lab[sel]
        lhs = np.zeros((KA, P), np.float32)
        lhs[0:D] = Fi[:, sel]
        lhs[D : D + L] = (lab_c[None, :] == lids[:, None]).astype(np.float32)
        lhs[D + L] = 1.0

        # partition-major transposed feature blocks [128, PB*D]
        fiT = np.zeros((128, PB * D), np.float32)
        fsT = np.zeros((128, PB * D), np.float32)
        Fsum = Fii[:, sel] + Fjj[:, sel]
        for b in range(PB):
            blk = slice(b * 128, (b + 1) * 128)
            fiT[:, b * D : (b + 1) * D] = Fi[:, sel][:, blk].T
            fsT[:, b * D : (b + 1) * D] = Fsum[:, blk].T

        ohlab = (lab_c[None, :] == lids[:, None]).astype(np.float32)  # [L, P]

        in_maps.append(
            {
                "lhs": lhs.astype(bf16),
                "rhs": rhs,
                "fiT": fiT,
                "fsT": fsT,
                "ohcnt": ohcnt,
                "ohlab": ohlab,
                "ones": ones,
            }
        )
    return in_maps


def kernel(features_i, features_ii, features_jj, i, ii, jj):
    global _compiled
    from concourse import bass_utils

    if _compiled is None:
        _compiled = _build()
    in_maps = _make_in_maps(features_i, features_ii, features_jj, i, ii, jj)
    results = bass_utils.run_bass_kernel_spmd(
        _compiled, in_maps, core_ids=list(range(NCORES))
    )
    total = np.float32(0.0)
    for r in results.results:
        total += np.float32(r["out"].reshape(-1)[0])
    return np.array(total, dtype=np.float32)
